# revision 9
# baseline (speedup 1.0000x reference)
"""AdderNet ResNet-20 forward on 8 Trainium2 NeuronCores (Bass/Tile).

Data parallel: batch 64 -> 8 images/core, weights replicated.

Each adder conv  out[o,n] = -sum_k |patch[k,n] - w[o,k]|  uses
    |d| = 2*max(d,0) - d   =>   out = -(2*M - P + W),  W = sum_k w[o,k].
Every adder conv is followed by BatchNorm, whose mean subtraction cancels
the per-channel constant W, so only  x_acc = P - 2*M  is materialized:

  - max(patch - w, 0): one fused DVE tensor_scalar (op0=subtract, op1=max)
    per (o-group, kernel tap), bf16 in/out (4x DVE mode), reading a
    zero-padded replicated input tile through a shifted access pattern.
  - k-reduction on TensorE: matmuls accumulating into PSUM; the lhsT
    column for output o holds -2 at that o-group's partitions.  P comes
    from matmuls on the raw input tile with lhsT = 1/rep.
  - BN batch stats: bn_stats/bn_aggr locally, one small AllGather per BN
    boundary, every core combines the 8 partial (mean, var) identically
    and applies y = relu(a*x + c) on ScalarE (per-partition a, c).

Input partition layout for a conv with Cin channels: partition (c, r) =
c*rep + r holds a copy of channel c's zero-padded image; the r slots let
one tensor_scalar handle `rep` output channels at once (their weights
differ per partition).
"""

import numpy as np


def _concourse():
    import sys
    try:
        import concourse  # noqa: F401
    except ImportError:
        for p in ("/opt/trn_rl_repo", "/root/.axon_site/_ro/trn_rl_repo"):
            if p not in sys.path:
                sys.path.insert(0, p)
    from concourse import bass, bacc, tile, mybir
    from concourse.bass_utils import run_bass_kernel_spmd
    return bass, bacc, tile, mybir, run_bass_kernel_spmd


N_CORES = 8
B_LOC = 8           # images per core
SC = 2048           # PSUM superchunk (output columns accumulated at once)
BN_EPS = 1e-5
REP = {3: 16, 16: 8, 32: 4, 64: 2}


# ---------------------------------------------------------------- network plan

class Conv:
    def __init__(self, name, cin, cout, hin, stride, k):
        self.name, self.cin, self.cout = name, cin, cout
        self.hin, self.stride, self.k = hin, stride, k
        self.rep = REP[cin]
        self.og = max(1, cout // self.rep)          # number of o-groups
        self.osz = cout // self.og                  # o's per group
        self.P = cin * self.rep                     # partitions of m tiles
        self.ho = hin // stride
        self.N = B_LOC * self.ho * self.ho
        self.hp = hin + 2                           # padded image side
        self.npad = B_LOC * self.hp * self.hp
        if k == 3:
            self.shifts = [(dy, dx) for dy in range(3) for dx in range(3)]
        else:                                       # 1x1 stride-2 on padded store
            self.shifts = [(1, 1)]
        self.wf_vec = None
        self.wb_m = None
        self.wb_p = None


def _chunk_geom(cfg, c512):
    """(b0, nb, y0, ny) covering output columns [c512*512, (c512+1)*512)."""
    howo = cfg.ho * cfg.ho
    if howo >= 512:
        per = howo // 512
        rows = 512 // cfg.ho
        return (c512 // per, 1, (c512 % per) * rows, rows)
    nimg = 512 // howo
    return (c512 * nimg, nimg, 0, cfg.ho)


class BN:
    def __init__(self, name, ch):
        self.name, self.ch = name, ch
        self.wf_gb = None


def make_plan():
    convs, bns = {}, {}

    def C(name, cin, cout, hin, stride, k=3):
        convs[name] = Conv(name, cin, cout, hin, stride, k)

    def Bn(name, ch):
        bns[name] = BN(name, ch)

    C("c0", 3, 16, 32, 1); Bn("bn0", 16)
    for i in range(3):
        C(f"l1b{i}c1", 16, 16, 32, 1); Bn(f"l1b{i}n1", 16)
        C(f"l1b{i}c2", 16, 16, 32, 1); Bn(f"l1b{i}n2", 16)
    C("l2b0c1", 16, 32, 32, 2); Bn("l2b0n1", 32)
    C("l2b0ds", 16, 32, 32, 2, k=1); Bn("l2b0nd", 32)
    C("l2b0c2", 32, 32, 16, 1); Bn("l2b0n2", 32)
    for i in (1, 2):
        C(f"l2b{i}c1", 32, 32, 16, 1); Bn(f"l2b{i}n1", 32)
        C(f"l2b{i}c2", 32, 32, 16, 1); Bn(f"l2b{i}n2", 32)
    C("l3b0c1", 32, 64, 16, 2); Bn("l3b0n1", 64)
    C("l3b0ds", 32, 64, 16, 2, k=1); Bn("l3b0nd", 64)
    C("l3b0c2", 64, 64, 8, 1); Bn("l3b0n2", 64)
    for i in (1, 2):
        C(f"l3b{i}c1", 64, 64, 8, 1); Bn(f"l3b{i}n1", 64)
        C(f"l3b{i}c2", 64, 64, 8, 1); Bn(f"l3b{i}n2", 64)
    Bn("bnfc", 10)

    f32c, b16c = 0, 0
    for cv in convs.values():
        cv.wf_vec = f32c; f32c += cv.og * len(cv.shifts)
        cv.wb_m = b16c;  b16c += cv.og * cv.cout
        cv.wb_p = b16c;  b16c += cv.cout
    for bn in bns.values():
        bn.wf_gb = f32c; f32c += 2
    fc_w = f32c; f32c += 10                 # fc weight vectors ([:64] rows)
    fc_l2 = f32c; f32c += 100               # fc lhsT blocks: j-th is [64,10], col j = -2
    return convs, bns, dict(f32_cols=f32c, b16_cols=b16c,
                            fc_w=fc_w, fc_l2=fc_l2)


def pack_weights(params, convs, bns, misc):
    import ml_dtypes
    wf = np.zeros((128, misc["f32_cols"]), np.float32)
    wb = np.zeros((128, misc["b16_cols"]), np.float32)

    def conv_w(cv, w):
        w = np.asarray(w, np.float32)       # [O, C, kh, kw]
        col = cv.wf_vec
        for og in range(cv.og):
            for (dy, dx) in cv.shifts:
                for c in range(cv.cin):
                    for r in range(cv.osz):
                        o = og * cv.osz + r
                        wf[c * cv.rep + r, col] = (
                            w[o, c, dy, dx] if cv.k == 3 else w[o, c, 0, 0])
                col += 1
        for og in range(cv.og):
            base = cv.wb_m + og * cv.cout
            for r in range(cv.osz):
                o = og * cv.osz + r
                for c in range(cv.cin):
                    wb[c * cv.rep + r, base + o] = -2.0
        wb[:cv.P, cv.wb_p: cv.wb_p + cv.cout] = 1.0 / cv.rep

    conv_w(convs["c0"], params["conv1_w"])
    gb = {"bn0": (params["bn1_g"], params["bn1_b"])}
    for li, lname in ((1, "layer1"), (2, "layer2"), (3, "layer3")):
        for i, p in enumerate(params[lname]):
            conv_w(convs[f"l{li}b{i}c1"], p["w1"])
            conv_w(convs[f"l{li}b{i}c2"], p["w2"])
            gb[f"l{li}b{i}n1"] = (p["g1"], p["b1"])
            gb[f"l{li}b{i}n2"] = (p["g2"], p["b2"])
            if "wd" in p:
                conv_w(convs[f"l{li}b{i}ds"], p["wd"])
                gb[f"l{li}b{i}nd"] = (p["gd"], p["bd"])
    gb["bnfc"] = (params["bn2_g"], params["bn2_b"])
    for name, (g, b) in gb.items():
        bn = bns[name]
        wf[:bn.ch, bn.wf_gb] = np.asarray(g, np.float32)
        wf[:bn.ch, bn.wf_gb + 1] = np.asarray(b, np.float32)

    fw = np.asarray(params["fc_w"], np.float32)     # [10, 64, 1, 1]
    for j in range(10):
        wf[:64, misc["fc_w"] + j] = fw[j, :, 0, 0]
        wf[:64, misc["fc_l2"] + j * 10 + j] = -2.0
    return wf, wb.astype(ml_dtypes.bfloat16)


# ---------------------------------------------------------------- device build

_CACHE = {}


def _build():
    if "nc" in _CACHE:
        return
    bass, bacc, tile, mybir, _run = _concourse()
    A = mybir.AluOpType
    F = mybir.ActivationFunctionType
    f32, bf16 = mybir.dt.float32, mybir.dt.bfloat16
    X = mybir.AxisListType.X

    convs, bns, misc = make_plan()
    _CACHE.update(convs=convs, bns=bns, misc=misc)

    import os
    dbg_on = bool(os.environ.get("ADDER_DEBUG"))
    nc = bacc.Bacc("TRN2", target_bir_lowering=False, debug=False,
                   num_devices=N_CORES)
    x_in = nc.dram_tensor("x", [B_LOC, 3, 32, 32], f32, kind="ExternalInput")
    wf_in = nc.dram_tensor("wf32", [128, misc["f32_cols"]], f32,
                           kind="ExternalInput")
    wb_in = nc.dram_tensor("wb16", [128, misc["b16_cols"]], bf16,
                           kind="ExternalInput")
    out_d = nc.dram_tensor("out", [10, B_LOC], f32, kind="ExternalOutput")

    RG = [list(range(N_CORES))]

    with tile.TileContext(nc) as tc:
        with tc.tile_pool(name="wpool", bufs=1) as wpool, \
             tc.tile_pool(name="repl", bufs=2) as replp, \
             tc.tile_pool(name="mtl", bufs=3) as mp, \
             tc.tile_pool(name="accp", bufs=1) as accp, \
             tc.tile_pool(name="ypool", bufs=4) as yp, \
             tc.tile_pool(name="vec", bufs=8) as vp, \
             tc.tile_pool(name="stat", bufs=4) as sp, \
             tc.tile_pool(name="psum", bufs=2, space="PSUM") as pp, \
             tc.tile_pool(name="dram", bufs=2, space="DRAM") as dp:

            wf = wpool.tile([128, misc["f32_cols"]], f32, tag="wf")
            wb = wpool.tile([128, misc["b16_cols"]], bf16, tag="wbt")
            nc.sync.dma_start(wf[:], wf_in[:])
            nc.sync.dma_start(wb[:], wb_in[:])
            onesl = wpool.tile([64, 10], f32, tag="fco")
            nc.vector.memset(onesl[:], 1.0)

            def replicate(ypad, cv):
                """ypad: [cin, npad] bf16 zero-padded -> replicated tile."""
                t = replp.tile([128, cv.npad], bf16, tag="repl")
                for r in range(cv.rep):
                    nc.sync.dma_start(t[r:cv.P:cv.rep, :], ypad[:cv.cin, :])
                return t

            def padded_tile(O, hp):
                """Fresh zero-initialized padded activation tile + interior view."""
                y = yp.tile([O, B_LOC * hp * hp], bf16, tag="y")
                nc.vector.memset(y[:], 0.0)
                hin = hp - 2
                iv = y[:].rearrange("p (b h w) -> p b h w",
                                    b=B_LOC, h=hp, w=hp)[:, :, 1:1 + hin,
                                                         1:1 + hin]
                return y, iv

            def window(repl_t, cv, b0, nb, y0, ny, dy, dx):
                v = repl_t[:cv.P, :].rearrange("p (b h w) -> p b h w",
                                               b=B_LOC, h=cv.hp, w=cv.hp)
                s = cv.stride
                return v[:, b0:b0 + nb,
                         dy + y0 * s: dy + (y0 + ny) * s: s,
                         dx: dx + cv.ho * s: s]

            def adder_conv(repl_t, cv, acc_tag="acc"):
                O = cv.cout
                acc = accp.tile([O, cv.N], f32, tag=acc_tag)
                scw = min(SC, cv.N)
                nsc = cv.N // scw
                howo = cv.ho * cv.ho
                for isc in range(nsc):
                    ps = pp.tile([O, scw], f32, tag="ps")
                    nch = scw // 512
                    for og in range(cv.og):
                        lhsT = wb[:cv.P,
                                  cv.wb_m + og * O: cv.wb_m + (og + 1) * O]
                        col = cv.wf_vec + og * len(cv.shifts)
                        for si, (dy, dx) in enumerate(cv.shifts):
                            m = mp.tile([cv.P, scw], bf16, tag="m")
                            src = window(repl_t, cv, isc * scw // howo,
                                         scw // howo, 0, cv.ho, dy, dx)
                            wvec = wf[:cv.P, col + si: col + si + 1]
                            nc.vector.tensor_scalar(m[:], src, wvec, 0.0,
                                                    op0=A.subtract, op1=A.max)
                            for ch in range(nch):
                                nc.tensor.matmul(
                                    ps[:, ch * 512:(ch + 1) * 512], lhsT,
                                    m[:, ch * 512:(ch + 1) * 512],
                                    start=(og == 0 and si == 0), stop=False)
                    lhsP = wb[:cv.P, cv.wb_p: cv.wb_p + O]
                    nshift = len(cv.shifts)
                    for si, (dy, dx) in enumerate(cv.shifts):
                        for ch in range(nch):
                            b0, nb, y0, ny = _chunk_geom(cv, isc * nch + ch)
                            src = window(repl_t, cv, b0, nb, y0, ny, dy, dx)
                            nc.tensor.matmul(
                                ps[:, ch * 512:(ch + 1) * 512], lhsP, src,
                                start=False,
                                stop=(si == nshift - 1) and (ch == nch - 1))
                    nc.scalar.copy(acc[:, isc * scw:(isc + 1) * scw], ps[:])
                return acc

            def local_stats(acc, O, N):
                nch = max(1, N // 512)
                st6 = sp.tile([O, nch * 6], f32, tag="st6")
                for j in range(nch):
                    nc.vector.bn_stats(st6[:, j * 6:(j + 1) * 6],
                                       acc[:, j * 512: min(N, (j + 1) * 512)])
                st2 = sp.tile([O, 2], f32, tag="st2")
                nc.vector.bn_aggr(st2[:], st6[:])
                return st2

            def bn_gather_apply(entries):
                """entries: [(st2, bn)]; one AllGather; returns [(a, c)]."""
                O = entries[0][1].ch
                k = 2 * len(entries)
                sb = sp.tile([O, k], f32, tag="ccsb")
                for i, (st2, bn) in enumerate(entries):
                    nc.vector.tensor_copy(sb[:bn.ch, 2 * i:2 * i + 2], st2[:])
                cin = dp.tile([O, k], f32, tag="ccin")
                nc.sync.dma_start(cin[:], sb[:])
                cout = dp.tile([N_CORES, O * k], f32, tag="ccout")
                nc.gpsimd.collective_compute(
                    "AllGather", A.bypass, replica_groups=RG,
                    ins=[cin.opt()], outs=[cout.opt()])
                g = sp.tile([O, N_CORES * k], f32, tag="ccg")
                nc.sync.dma_start(
                    g[:].rearrange("o (r k) -> o r k", r=N_CORES, k=k),
                    cout[:].rearrange("r (o k) -> o r k", o=O, k=k))
                res = []
                for i, (_, bn) in enumerate(entries):
                    ch = bn.ch
                    mcol = g[:ch, 2 * i::k]                 # [ch, 8] stride k
                    vcol = g[:ch, 2 * i + 1::k]
                    mu = vp.tile([ch, 1], f32, tag="v_mu")
                    nc.vector.reduce_sum(mu[:], mcol, axis=X)
                    nc.vector.tensor_scalar(mu[:], mu[:], 1.0 / N_CORES, None,
                                            op0=A.mult)
                    var = vp.tile([ch, 1], f32, tag="v_var")
                    nc.vector.reduce_sum(var[:], vcol, axis=X)
                    msq = vp.tile([ch, N_CORES], f32, tag="v_msq")
                    nc.vector.tensor_tensor(msq[:], mcol, mcol, op=A.mult)
                    msqs = vp.tile([ch, 1], f32, tag="v_msqs")
                    nc.vector.reduce_sum(msqs[:], msq[:], axis=X)
                    nc.vector.tensor_tensor(var[:], var[:], msqs[:], op=A.add)
                    nc.vector.tensor_scalar(var[:], var[:], 1.0 / N_CORES,
                                            None, op0=A.mult)
                    mu2 = vp.tile([ch, 1], f32, tag="v_mu2")
                    nc.vector.tensor_tensor(mu2[:], mu[:], mu[:], op=A.mult)
                    nc.vector.tensor_tensor(var[:], var[:], mu2[:],
                                            op=A.subtract)
                    t = vp.tile([ch, 1], f32, tag="v_t")
                    nc.vector.tensor_scalar(t[:], var[:], BN_EPS, None,
                                            op0=A.add)
                    s = vp.tile([ch, 1], f32, tag="v_s")
                    nc.scalar.sqrt(s[:], t[:])
                    r = vp.tile([ch, 1], f32, tag="v_r")
                    nc.vector.reciprocal(r[:], s[:])
                    q = vp.tile([ch, 1], f32, tag="v_q")
                    nc.vector.tensor_tensor(q[:], t[:], r[:], op=A.mult)
                    nc.vector.tensor_tensor(s[:], s[:], q[:], op=A.add)
                    nc.vector.tensor_scalar(s[:], s[:], 0.5, None, op0=A.mult)
                    inv = vp.tile([ch, 1], f32, tag="v_inv")
                    nc.vector.reciprocal(inv[:], s[:])
                    a = vp.tile([ch, 1], f32, tag="v_a")
                    nc.vector.tensor_tensor(a[:], wf[:ch, bn.wf_gb:bn.wf_gb + 1],
                                            inv[:], op=A.mult)
                    c = vp.tile([ch, 1], f32, tag="v_c")
                    nc.vector.tensor_tensor(c[:], a[:], mu[:], op=A.mult)
                    nc.vector.tensor_tensor(
                        c[:], wf[:ch, bn.wf_gb + 1:bn.wf_gb + 2], c[:],
                        op=A.subtract)
                    res.append((a, c))
                return res

            def apply_bn(acc, a, c, O, ho, relu):
                """BN(+relu) from acc [O, N] into a zero-padded bf16 tile."""
                y, iv = padded_tile(O, ho + 2)
                av = acc[:].rearrange("p (b h w) -> p b h w", b=B_LOC,
                                      h=ho, w=ho)
                nc.scalar.activation(iv, av,
                                     F.Relu if relu else F.Identity,
                                     bias=c[:], scale=a[:])
                return y

            # ---------------- network ----------------
            xs = accp.tile([3, B_LOC * 1024], f32, tag="acc")
            nc.sync.dma_start(
                xs[:].rearrange("c (b n) -> c b n", b=B_LOC),
                x_in[:].rearrange("b c h w -> c b (h w)"))
            y, iv0 = padded_tile(3, 34)
            nc.vector.tensor_copy(
                iv0, xs[:].rearrange("p (b h w) -> p b h w", b=B_LOC,
                                     h=32, w=32))

            def dbg_tap(name, ap, shape, dtype):
                if not dbg_on:
                    return
                t = nc.dram_tensor(name, shape, dtype, kind="ExternalOutput")
                nc.sync.dma_start(t[:], ap)

            cv = convs["c0"]
            rp = replicate(y, cv)
            dbg_tap("dbg_y0", y[:], [3, 9248], bf16)
            dbg_tap("dbg_repl", rp[:], [128, 9248], bf16)
            acc = adder_conv(rp, cv)
            dbg_tap("dbg_acc0", acc[:], [16, 8192], f32)
            st = local_stats(acc, 16, cv.N)
            dbg_tap("dbg_st2", st[:], [16, 2], f32)
            (a, c), = bn_gather_apply([(st, bns["bn0"])])
            dbg_tap("dbg_a", a[:], [16, 1], f32)
            dbg_tap("dbg_c", c[:], [16, 1], f32)
            y = apply_bn(acc, a, c, 16, cv.ho, True)
            dbg_tap("dbg_y1", y[:], [16, 9248], bf16)

            for li in (1, 2, 3):
                for bi in range(3):
                    pref = f"l{li}b{bi}"
                    c1 = convs[pref + "c1"]
                    c2 = convs[pref + "c2"]
                    has_ds = (pref + "ds") in convs
                    rin = replicate(y, c1)
                    acc1 = adder_conv(rin, c1)
                    st1 = local_stats(acc1, c1.cout, c1.N)
                    entries = [(st1, bns[pref + "n1"])]
                    if has_ds:
                        ds = convs[pref + "ds"]
                        accd = adder_conv(rin, ds, acc_tag="accd")
                        std = local_stats(accd, ds.cout, ds.N)
                        entries.append((std, bns[pref + "nd"]))
                    acs = bn_gather_apply(entries)
                    a1, cc1 = acs[0]
                    y1 = apply_bn(acc1, a1, cc1, c1.cout, c1.ho, True)
                    if has_ds:
                        ad, cd = acs[1]
                        idn = apply_bn(accd, ad, cd, ds.cout, c1.ho, False)
                    else:
                        idn = y
                    r2 = replicate(y1, c2)
                    acc2 = adder_conv(r2, c2)
                    stx = local_stats(acc2, c2.cout, c2.N)
                    (a2, c2v), = bn_gather_apply([(stx, bns[pref + "n2"])])
                    t2 = apply_bn(acc2, a2, c2v, c2.cout, c2.ho, False)
                    ho = c2.ho
                    z, ziv = padded_tile(c2.cout, ho + 2)
                    def _iview(tt):
                        return tt[:].rearrange(
                            "p (b h w) -> p b h w", b=B_LOC, h=ho + 2,
                            w=ho + 2)[:, :, 1:1 + ho, 1:1 + ho]
                    nc.vector.tensor_tensor(ziv, _iview(t2), _iview(idn),
                                            op=A.add)
                    nc.vector.tensor_scalar(ziv, ziv, 0.0, None, op0=A.max)
                    y = z

            # ---------------- avgpool + fc + final bn ----------------
            pooled = sp.tile([64, B_LOC], f32, tag="pool")
            yfv = y[:].rearrange("p (b h w) -> p b h w", b=B_LOC, h=10,
                                 w=10)[:, :, 1:9, 1:9]
            nc.vector.reduce_sum(pooled[:], yfv, axis=mybir.AxisListType.XY)
            nc.vector.tensor_scalar(pooled[:], pooled[:], 1.0 / 64, None,
                                    op0=A.mult)
            psfc = pp.tile([10, B_LOC], f32, tag="ps")
            for j in range(10):
                mj = sp.tile([64, B_LOC], f32, tag="mfc")
                wvec = wf[:64, misc["fc_w"] + j: misc["fc_w"] + j + 1]
                nc.vector.tensor_scalar(mj[:], pooled[:], wvec, 0.0,
                                        op0=A.subtract, op1=A.max)
                l2 = wf[:64, misc["fc_l2"] + j * 10: misc["fc_l2"] + (j + 1) * 10]
                nc.tensor.matmul(psfc[:], l2, mj[:], start=(j == 0), stop=False)
            nc.tensor.matmul(psfc[:], onesl[:], pooled[:],
                             start=False, stop=True)
            accf = sp.tile([10, B_LOC], f32, tag="accf")
            nc.scalar.copy(accf[:], psfc[:])
            stf = local_stats(accf, 10, B_LOC)
            (af, cf), = bn_gather_apply([(stf, bns["bnfc"])])
            outf = sp.tile([10, B_LOC], f32, tag="outf")
            nc.scalar.activation(outf[:], accf[:], F.Identity,
                                 bias=cf[:], scale=af[:])
            nc.sync.dma_start(out_d[:], outf[:])

    nc.compile()
    _CACHE["nc"] = nc
    _CACHE["run"] = _run


def kernel(x, params):
    _build()
    convs, bns, misc = _CACHE["convs"], _CACHE["bns"], _CACHE["misc"]
    wf, wb16 = pack_weights(params, convs, bns, misc)
    x = np.asarray(x, np.float32)
    in_maps = [{
        "x": np.ascontiguousarray(x[i * B_LOC:(i + 1) * B_LOC]),
        "wf32": wf,
        "wb16": wb16,
    } for i in range(N_CORES)]
    res = _CACHE["run"](_CACHE["nc"], in_maps, core_ids=list(range(N_CORES)))
    out = np.concatenate([res.results[i]["out"].T for i in range(N_CORES)], 0)
    return np.ascontiguousarray(out, np.float32)
